# revision 28
# baseline (speedup 1.0000x reference)
"""Multi-head attention (B=2, S=2048, D=1024, H=16, dk=64) on 8 trn2 cores.

Sharding: batch (2) x head-group (4 heads each) = 8 shards.
Core c handles batch b = c // 4, heads g = c % 4 (heads 4g..4g+3).

Host-side prep per core (unchanged from v1):
  - inputs transposed to [d, s] so the contraction dim lands on SBUF
    partitions with no on-chip transposes,
  - Wq/Wk/Wv column-sharded per head group (1/sqrt(dk) folded into Wq/bq),
  - Wo row-sharded, transposed to [c, j],
  - each core emits a transposed partial output [1024, 2048]; host sums the
    4 partials per batch (bo/4 folded into each partial) and transposes back.

On-chip dataflow v2 (per core) - differences vs v1:
  - attn@V runs with au (=exp scores) as the STATIONARY operand and V as the
    moving operand: out[q, dv'] accumulates over k-tiles.  Matmul cost on the
    PE is (moving columns) per instruction, so this costs 65 cols per k-tile
    instead of 512, cutting attn@V PE time in half vs v1.
  - the denominator lands in column 64 of the same [q, 65] PSUM tile (ones
    column in V'), so softmax normalization is a per-partition reciprocal +
    tensor_scalar multiply on the DVE - the v1 PE-broadcast contraption
    (K=1 matmuls of a hi/lo bf16 split) is gone.
  - attention output is produced in [q, c] layout; a cheap PE transpose
    (identity matmul) flips it to [c, q] for the output projection.
  - instruction emission interleaves projection / attn@V / transpose /
    out-proj matmuls between score-matmul rounds so the PE and Act engines
    (exp is ~133us of Act time) both stay busy; score PSUM rotates through
    2 double-buffered 2-slot tiles, attn@V and general work each get 2
    more PSUM banks (8 banks exactly).

All matmuls run in bf16 (fp32 PSUM accumulate).  Full-kernel relative error
vs the fp32 reference is ~6e-3.
"""

from collections import deque
from contextlib import ExitStack

import ml_dtypes
import numpy as np

import concourse.bacc as bacc
import concourse.mybir as mybir
import concourse.tile as tile
from concourse.bass_utils import run_bass_kernel_spmd

F32 = mybir.dt.float32
BF16 = mybir.dt.bfloat16

D = 1024          # d_model
S = 2048          # sequence length
HCORE = 4         # heads per core
DK = 64           # head dim
M = HCORE * DK    # 256 sharded projection width
P = 128

N_CORES = 8
ST = 512          # s-tile (free dim of projection / q-tile)
N_ST = S // ST    # 4
N_DT = D // P     # 8 contraction tiles for projections
N_KT = S // P     # 16 k-tiles for attention
N_JT = D // P     # 8 output row tiles
N_PAIR = N_ST * HCORE   # 16 (qt, h) pairs
N_RND = 8         # score rounds per pair, 2 k-tiles each


def build_mha_tile(tc, outs, ins):
    nc = tc.nc
    xqT, xkT, xvT = ins["xqT"], ins["xkT"], ins["xvT"]
    wq, wk, wv, woT = ins["wq"], ins["wk"], ins["wv"], ins["woT"]
    bq, bk, bvb, bo4 = ins["bq"], ins["bk"], ins["bvb"], ins["bo4"]
    ident = ins["ident"]
    outT = outs["outT"]

    ctx = ExitStack()
    ec = ctx.enter_context
    const = ec(tc.tile_pool(name="const", bufs=1))
    persist = ec(tc.tile_pool(name="persist", bufs=1))
    xin_pool = ec(tc.tile_pool(name="xin", bufs=9))
    au_pool = ec(tc.tile_pool(name="au", bufs=26))
    ob_pool = ec(tc.tile_pool(name="outb", bufs=2))
    small = ec(tc.tile_pool(name="small", bufs=4))
    sc_ps = ec(tc.tile_pool(name="sc_ps", bufs=2, space="PSUM"))
    av_ps = ec(tc.tile_pool(name="av_ps", bufs=2, space="PSUM"))
    pp_ps = ec(tc.tile_pool(name="pp_ps", bufs=2, space="PSUM"))

    xqT3 = xqT.rearrange("(dt p) s -> p dt s", p=P)
    xkT3 = xkT.rearrange("(dt p) s -> p dt s", p=P)
    xvT3 = xvT.rearrange("(dt p) s -> p dt s", p=P)
    outT3 = outT.rearrange("(jt p) s -> p jt s", p=P)

    # ---- persistent activations ----
    QT_sb = persist.tile([P, 2, S], BF16)          # [p, mt, s]
    KT_sb = persist.tile([P, 2, S], BF16)
    V_sb = persist.tile([P, N_KT, HCORE, DK + 1], BF16)   # [p, kt, h, dv']
    cat_sb = persist.tile([P, N_KT, M], BF16)      # attn out [q-part, qtile, c]
    catT_sb = persist.tile([P, 2, S], BF16)        # transposed [c-part, ct, q]

    nc.vector.memset(V_sb[:, :, :, DK], 1.0)      # ones column for denominators

    # PE p-state warmup: ~3us of dummy matmuls on a memset tile so the ramp
    # to full clock is over before the first projection data lands
    spin = persist.tile([P, ST], BF16, name="spin")
    nc.gpsimd.memset(spin, 0.0)
    for i in range(8):
        ps = pp_ps.tile([P, ST], F32, tag="pp", name="spin_ps")
        nc.tensor.matmul(ps, spin[:, 0:P], spin, start=True, stop=True)

    # The `warm` DVE touches absorb each bias tile's DMA-lane wait so it
    # never lands as a 2nd sync wait on a hot DVE op (walrus allows only 1).
    warm = const.tile([P, 16], F32)

    # ---- const + input stream DMAs over three parallel issue queues (SP,
    # Activation, gpsimd/SWDGE) so the K st0 / Q st0 projections that gate
    # the first exp are not serialized behind each other's transfers.
    # Weights are loaded in mt-halves: only the mt0 halves gate pair 0. ----
    xin = {}

    def load_x(eng, key, src3, st, split=False, interleave_with=None):
        t = xin_pool.tile([P, N_DT, ST], BF16, tag="xt", name="xt")
        if split:
            # two half transfers so the first projection granule can start
            # after only 0.5MB has landed; optionally interleave another
            # split load between the halves (xk0a, xq0a, xk0b, xq0b)
            h = ST // 2
            eng.dma_start(t[:, :, 0:h], src3[:, :, st * ST:st * ST + h])
            xin[(key, st)] = t
            if interleave_with is not None:
                interleave_with()
            eng.dma_start(t[:, :, h:ST], src3[:, :, st * ST + h:(st + 1) * ST])
        else:
            eng.dma_start(t, src3[:, :, st * ST:(st + 1) * ST])
            xin[(key, st)] = t

    # All transfers serialize on the one modeled DMA device (~2.9us/MB), so
    # a single SP-queue stream in exact need order IS the transfer schedule:
    # Q/K st0 halves first (gate the first exp), then K st1-3 (score rounds
    # of pair 0), the mt1 weight halves (pair 2), V path, then out-proj
    # constants.  xq st1-3 ride the gpsimd queue later (they reuse xk bufs).
    wk_sb = [const.tile([P, N_DT, P], BF16, name=f"wk{m}_sb") for m in range(2)]
    wq_sb = [const.tile([P, N_DT, P], BF16, name=f"wq{m}_sb") for m in range(2)]
    nc.sync.dma_start(wk_sb[0], wk[0])
    nc.sync.dma_start(wq_sb[0], wq[0])
    load_x(nc.sync, "k", xkT3, 0, split=True, interleave_with=lambda: load_x(
        nc.sync, "q", xqT3, 0, split=True))
    bk_sb = const.tile([P, 2], F32, name="bk_sb")
    nc.sync.dma_start(bk_sb, bk)
    nc.vector.tensor_copy(warm[:, 2:4], bk_sb)
    bq_sb = const.tile([P, 2], F32, name="bq_sb")
    nc.sync.dma_start(bq_sb, bq)
    nc.vector.tensor_copy(warm[:, 0:2], bq_sb)
    for st in range(1, N_ST):
        load_x(nc.sync, "k", xkT3, st)
    wv_sb = const.tile([P, N_DT, M], BF16, name="wv_sb")
    nc.sync.dma_start(wv_sb, wv)
    bvb_sb = const.tile([P, M], F32, name="bvb_sb")
    nc.sync.dma_start(bvb_sb, bvb)
    nc.vector.tensor_copy(warm[:, 12:16], bvb_sb[:, 0:4])
    nc.sync.dma_start(wk_sb[1], wk[1])
    nc.sync.dma_start(wq_sb[1], wq[1])
    for st in range(N_ST):
        load_x(nc.sync, "v", xvT3, st)
    id_sb = const.tile([P, P], BF16, name="id_sb")
    nc.sync.dma_start(id_sb, ident)
    bo4_sb = const.tile([P, N_JT], F32, name="bo4_sb")
    nc.sync.dma_start(bo4_sb, bo4)
    nc.vector.tensor_copy(warm[:, 4:12], bo4_sb)
    woT_sb = const.tile([P, 2, D], BF16, name="woT_sb")
    nc.sync.dma_start(woT_sb, woT)

    # ---- emission helpers ----
    HS = ST // 2      # half-ST projection granule (2048 PE cycles)

    def proj_qk_mt(key, w_sb, b_sb, dst_sb, st, mt, half):
        """dst[:, mt, half-st-slice] = w.T @ xT + b, one 2048-cycle granule."""
        xt = xin[(key, st)]
        hs = slice(half * HS, (half + 1) * HS)
        ps = pp_ps.tile([P, ST], F32, tag="pp", name="qk_ps")[:, :HS]
        for dt in range(N_DT):
            nc.tensor.matmul(
                ps,
                w_sb[mt][:, dt, :],
                xt[:, dt, hs],
                start=(dt == 0), stop=(dt == N_DT - 1))
        nc.vector.tensor_scalar_add(
            dst_sb[:, mt, st * ST + half * HS:st * ST + (half + 1) * HS],
            ps, b_sb[:, mt:mt + 1])

    def v_chunk(h, ktg):
        """V[kt-rows, h, dv] for k-tiles 4*ktg..4*ktg+3, one head only, so
        attn@V of head h only ever waits for its own quarter of V-proj."""
        xt = xin[("v", ktg)]
        for kl in range(4):
            kt = 4 * ktg + kl
            ps = pp_ps.tile([P, ST], F32, tag="pp", name="v_ps")[:, :DK]
            for dt in range(N_DT):
                nc.tensor.matmul(
                    ps, xt[:, dt, kl * P:(kl + 1) * P],
                    wv_sb[:, dt, h * DK:(h + 1) * DK],
                    start=(dt == 0), stop=(dt == N_DT - 1))
            nc.vector.tensor_add(
                out=V_sb[:, kt, h, 0:DK],
                in0=ps, in1=bvb_sb[:, h * DK:(h + 1) * DK])

    def transp_chunk(qt, ct):
        """catT[:, ct, qt-block] = cat[qt-block, ct-block].T via PE."""
        for qtg in range(4):
            qq = qt * 4 + qtg
            tp = pp_ps.tile([P, 1024], BF16, tag="pp", name="tp_ps")[:, :P]
            nc.tensor.transpose(
                tp, cat_sb[:, qq, ct * P:(ct + 1) * P], id_sb)
            nc.vector.tensor_copy(
                catT_sb[:, ct, qq * P:(qq + 1) * P], tp)

    ob_tiles = {}

    def outp_chunk(qt, jt):
        """outT[jt, qt-slice] = woT.T @ catT + bo/4, DMA at jt 3 and 7."""
        qs = slice(qt * ST, (qt + 1) * ST)
        if jt == 0:
            ob_tiles[qt] = ob_pool.tile([P, N_JT, ST], BF16, tag="ob",
                                        name="ob")
        ob = ob_tiles[qt]
        ps = pp_ps.tile([P, ST], F32, tag="pp", name="op_ps")
        for ct in range(2):
            nc.tensor.matmul(
                ps,
                woT_sb[:, ct, jt * P:(jt + 1) * P],
                catT_sb[:, ct, qs],
                start=(ct == 0), stop=(ct == 1))
        if qt == N_ST - 1 and jt % 2 == 0:
            # the exp stream is over by now: the idle Act engine takes half
            # the bias-adds so the tail is PE-bound instead of DVE-bound
            nc.scalar.activation(
                ob[:, jt, :], ps, mybir.ActivationFunctionType.Identity,
                bias=bo4_sb[:, jt:jt + 1])
        else:
            nc.vector.tensor_scalar_add(ob[:, jt, :], ps, bo4_sb[:, jt:jt + 1])
        if qt == N_ST - 1:
            # final tile: per-jt DMA so the drain only waits on one jt slab
            nc.sync.dma_start(outT3[:, jt:jt + 1, qs], ob[:, jt:jt + 1, :])
        elif jt == N_JT // 2 - 1:
            nc.sync.dma_start(outT3[:, :N_JT // 2, qs], ob[:, :N_JT // 2, :])
        elif jt == N_JT - 1:
            nc.sync.dma_start(outT3[:, N_JT // 2:, qs], ob[:, N_JT // 2:, :])

    # ---- filler scheduler ----
    round_clock = [0]      # sc rounds emitted so far (proxy for Act time)
    # xv chunk q becomes usable at ~(24.6 + 2.9q)us on the serial DMA device;
    # in Act rounds (first exp ~13us, 1.04us/round) that is round ~11+3q.
    VREADY = [13, 16, 19, 22]
    fillers = deque()
    remaining = {}

    def enq(key, cycles, fn):
        fillers.append((key, cycles, fn))
        remaining[key] = remaining.get(key, 0) + 1

    def run_next():
        key, cy, fn = fillers.popleft()
        fn()
        remaining[key] -= 1
        return cy

    def v_gated(key):
        return (key[0] == "V" and round_clock[0] < VREADY[key[2]])

    def fill(budget):
        # pop filler granules, skipping V-proj chunks whose xvT slab is
        # still in flight on the serial DMA device (they would head-of-line
        # block the PE); V chunks have no ordering ties to other entries
        while fillers and budget > 0:
            for i, (key, cy, fn) in enumerate(fillers):
                if not v_gated(key):
                    fillers.rotate(-i)
                    budget -= run_next()
                    fillers.rotate(i)
                    break
            else:
                return

    def drain_until(key):
        while remaining.get(key, 0) > 0:
            run_next()

    def enq_k(st, mt):
        for half in range(2):
            enq(("K", st, mt), 2048,
                lambda half=half: proj_qk_mt("k", wk_sb, bk_sb, KT_sb,
                                             st, mt, half))

    def enq_q(st, mt):
        for half in range(2):
            enq(("Q", st, mt), 2048,
                lambda half=half: proj_qk_mt("q", wq_sb, bq_sb, QT_sb,
                                             st, mt, half))

    def enq_v(h):
        for ktg in range(4):
            enq(("V", h, ktg), 2048, lambda h=h, ktg=ktg: v_chunk(h, ktg))

    def enq_xq(st):
        # late xq loads reuse xk bufs; emitted only after the matching
        # ("K", st-1, 1) filler so the WAR on the old tile is tracked
        enq(("D", st), 0, lambda st=st: load_x(nc.gpsimd, "q", xqT3, st))

    # mt0 chunks feed heads 0/1 (pairs 0-1), mt1 chunks heads 2/3 (pairs 2-3)
    for st in range(1, N_ST):
        enq_k(st, 0)
    enq_q(0, 1)
    enq_k(0, 1)
    enq_k(1, 1)
    enq_xq(1)
    enq_k(2, 1)
    enq_xq(2)
    enq_k(3, 1)
    enq_xq(3)
    enq_v(0)
    enq_v(1)
    enq_q(1, 0)
    enq_q(1, 1)
    enq_v(2)
    enq_q(2, 0)
    enq_q(2, 1)
    enq_v(3)
    enq_q(3, 0)
    enq_q(3, 1)

    # ---- attention emission ----
    def sc_round(qt, h, r):
        """Scores for k-tiles (2r, 2r+1) -> exp -> au tile [P, 2, ST]."""
        mt, p0 = h // 2, (h % 2) * DK
        qs = slice(qt * ST, (qt + 1) * ST)
        round_clock[0] += 1
        sc = sc_ps.tile([P, 2, ST], F32, tag="sc")
        au = au_pool.tile([P, 2, ST], BF16, tag="au")
        for i in range(2):
            kt = 2 * r + i
            nc.tensor.matmul(
                sc[:, i, :],
                KT_sb[p0:p0 + DK, mt, kt * P:(kt + 1) * P],
                QT_sb[p0:p0 + DK, mt, qs],
                start=True, stop=True)
        nc.scalar.activation(au, sc, mybir.ActivationFunctionType.Exp)
        return au

    # attn@V runs off a quarter-granular work queue: each item is 4 k-tiles
    # of one (pair, qtile) accumulation chain.  The queue self-paces against
    # V-projection availability (DMA-ordered) and naturally pipelines the
    # final pair's chains against its own exp stream.
    avq = deque()
    av_open = {}
    chains_done = {}

    def avq_append_pair(p, qt, h, au_tiles, qtgs=(0, 1, 2, 3),
                        interleave=False):
        if interleave:
            for quarter in range(4):
                for qtg in qtgs:
                    avq.append((qt, h, au_tiles, qtg, quarter))
        else:
            for qtg in qtgs:
                for quarter in range(4):
                    avq.append((qt, h, au_tiles, qtg, quarter))

    def avq_step(n):
        """Emit up to n attn@V quarters (~260 cycles each)."""
        done = 0
        while avq and done < n:
            qt, h, au_tiles, qtg, quarter = avq[0]
            if len(au_tiles) < 2 * (quarter + 1):
                break          # exp for these k-tiles not emitted yet
            if round_clock[0] < VREADY[quarter]:
                break          # V-proj input for these k-tiles still in DMA
            avq.popleft()
            drain_until(("V", h, quarter))
            if quarter == 0:
                ps = av_ps.tile([P, ST], F32, tag="av", name="av_ps")[:, :DK + 1]
                av_open[(qt, h, qtg)] = ps
            else:
                ps = av_open[(qt, h, qtg)]
            q0 = qtg * P
            for kl in range(4):
                kt = 4 * quarter + kl
                nc.tensor.matmul(
                    ps,
                    au_tiles[kt // 2][:, kt % 2, q0:q0 + P],
                    V_sb[:, kt, h, :],
                    start=(kt == 0), stop=(kt == N_KT - 1))
            if quarter == 3:
                av_open.pop((qt, h, qtg))
                recip = small.tile([P, 1], F32, tag="recip")
                nc.vector.reciprocal(recip, ps[:, DK:DK + 1])
                nc.vector.tensor_scalar_mul(
                    cat_sb[:, qt * 4 + qtg, h * DK:(h + 1) * DK],
                    ps[:, 0:DK], recip)
                nd = chains_done[(qt, h)] = chains_done.get((qt, h), 0) + 1
                if nd == 4:
                    # this pair's attention output slab is complete: its
                    # transpose (and out-proj once all heads are in) can go
                    if h == 1:
                        enq(("T", qt, 0), 512,
                            lambda qt=qt: transp_chunk(qt, 0))
                    elif h == 3:
                        enq(("T", qt, 1), 512,
                            lambda qt=qt: transp_chunk(qt, 1))
                        for jt in range(N_JT):
                            enq(("O", qt, jt), 1024,
                                lambda qt=qt, jt=jt: outp_chunk(qt, jt))
            done += 1

    # pre-pair work: only the mt0 halves of K st0 / Q st0 gate pair 0,
    # emitted in DMA-arrival order (xk0a, xq0a, xq0b, xk0b)
    proj_qk_mt("k", wk_sb, bk_sb, KT_sb, 0, 0, 0)
    proj_qk_mt("q", wq_sb, bq_sb, QT_sb, 0, 0, 0)
    proj_qk_mt("q", wq_sb, bq_sb, QT_sb, 0, 0, 1)
    proj_qk_mt("k", wk_sb, bk_sb, KT_sb, 0, 0, 1)

    for p in range(N_PAIR):
        qt, h = divmod(p, HCORE)
        mt = h // 2
        drain_until(("Q", qt, mt))
        last = p == N_PAIR - 1
        au_tiles = []
        for r in range(N_RND):
            drain_until(("K", r // 2, mt))
            au_tiles.append(sc_round(qt, h, r))
            # lookahead drains: pull future deps in small steps so pair/round
            # boundaries never force a large burst of queued filler work
            if p == 1 and r % 2 == 0:
                drain_until(("K", r // 2, 1))
            if h == 1 and r == 4:
                drain_until(("Q", qt, 1))
            if h == 3 and r == 4 and qt < 3:
                drain_until(("Q", qt + 1, 0))
            if last and r == 1:
                # feed this pair's first two chains as its own exps land
                avq_append_pair(p, qt, h, au_tiles, qtgs=(0, 1),
                                interleave=True)
            backlog = len(avq)
            avq_step(4 if last else (3 if backlog > 20 else 2))
            fill(1300 if p >= 13 else (2000 if p == 0 else 900))
        if not last:
            avq_append_pair(p, qt, h, au_tiles)
        else:
            avq_append_pair(p, qt, h, au_tiles, qtgs=(2, 3))

    # ---- tail: drain remaining attn@V quarters, then final out-proj ----
    while avq:
        avq_step(4)
        fill(1000)
    while fillers:
        run_next()
    ctx.close()


def build_bass():
    nc = bacc.Bacc(trn_type="TRN2", target_bir_lowering=False, debug=False)
    ins = {
        "xqT": nc.dram_tensor("xqT", (D, S), BF16, kind="ExternalInput").ap(),
        "xkT": nc.dram_tensor("xkT", (D, S), BF16, kind="ExternalInput").ap(),
        "xvT": nc.dram_tensor("xvT", (D, S), BF16, kind="ExternalInput").ap(),
        "wq": nc.dram_tensor("wq", (2, P, N_DT, P), BF16, kind="ExternalInput").ap(),
        "wk": nc.dram_tensor("wk", (2, P, N_DT, P), BF16, kind="ExternalInput").ap(),
        "wv": nc.dram_tensor("wv", (P, N_DT, M), BF16, kind="ExternalInput").ap(),
        "woT": nc.dram_tensor("woT", (P, 2, D), BF16, kind="ExternalInput").ap(),
        "bq": nc.dram_tensor("bq", (P, 2), F32, kind="ExternalInput").ap(),
        "bk": nc.dram_tensor("bk", (P, 2), F32, kind="ExternalInput").ap(),
        "bvb": nc.dram_tensor("bvb", (P, M), F32, kind="ExternalInput").ap(),
        "bo4": nc.dram_tensor("bo4", (P, N_JT), F32, kind="ExternalInput").ap(),
        "ident": nc.dram_tensor("ident", (P, P), BF16, kind="ExternalInput").ap(),
    }
    outs = {
        "outT": nc.dram_tensor("outT", (D, S), BF16, kind="ExternalOutput").ap(),
    }
    with tile.TileContext(nc) as tc:
        build_mha_tile(tc, outs, ins)
    nc.compile()
    return nc


def shard_inputs(query, key, value, Wq, bq, Wk, bk, Wv, bv, Wo, bo):
    """Build the 8 per-core input maps (all host-side numpy layout prep)."""
    def prep_w(W, ms, scale=1.0, mt_major=False):
        # [d, m] -> [p, dt, m]  (or mt-major [2, p, dt, 128] for split loads)
        wT = (np.asarray(W)[ms, :].T * scale).astype(ml_dtypes.bfloat16)
        w = wT.reshape(N_DT, P, M).transpose(1, 0, 2)
        if mt_major:
            w = np.stack([w[:, :, 0:P], w[:, :, P:M]])
        return np.ascontiguousarray(w)

    def prep_b(b, ms, scale=1.0):
        return np.ascontiguousarray(
            (np.asarray(b)[ms] * scale).astype(np.float32).reshape(2, P).T)

    ident = np.eye(P, dtype=ml_dtypes.bfloat16)
    in_maps = []
    for c in range(N_CORES):
        b_idx, g = divmod(c, N_CORES // 2)
        ms = slice(g * M, (g + 1) * M)
        woT = np.ascontiguousarray(Wo[:, ms].T.astype(np.float32))
        in_maps.append({
            "xqT": np.ascontiguousarray(query[b_idx].T.astype(ml_dtypes.bfloat16)),
            "xkT": np.ascontiguousarray(key[b_idx].T.astype(ml_dtypes.bfloat16)),
            "xvT": np.ascontiguousarray(value[b_idx].T.astype(ml_dtypes.bfloat16)),
            "wq": prep_w(Wq, ms, 1.0 / np.sqrt(DK), mt_major=True),
            "wk": prep_w(Wk, ms, mt_major=True),
            "wv": prep_w(Wv, ms),
            "woT": np.ascontiguousarray(
                woT.astype(ml_dtypes.bfloat16).reshape(2, P, D).transpose(1, 0, 2)),
            "bq": prep_b(bq, ms, 1.0 / np.sqrt(DK)),
            "bk": prep_b(bk, ms),
            "bvb": np.ascontiguousarray(
                np.tile(np.asarray(bv)[ms].astype(np.float32), (P, 1))),
            "bo4": np.ascontiguousarray(
                (np.asarray(bo) / (N_CORES // 2)).astype(np.float32)
                .reshape(N_JT, P).T),
            "ident": ident,
        })
    return in_maps


_NC_CACHE = None
_RUNNER_CACHE = None


def _get_nc():
    global _NC_CACHE
    if _NC_CACHE is None:
        _NC_CACHE = build_bass()
    return _NC_CACHE


def _axon_runner():
    """Jit the SPMD NEFF exec once (no donation; kernel writes every output
    element, so reusing non-donated zero buffers across calls is safe)."""
    global _RUNNER_CACHE
    if _RUNNER_CACHE is not None:
        return _RUNNER_CACHE
    import jax
    from jax.experimental.shard_map import shard_map
    from jax.sharding import Mesh, PartitionSpec
    from concourse.bass2jax import (_bass_exec_p, install_neuronx_cc_hook,
                                    partition_id_tensor)

    nc = _get_nc()
    install_neuronx_cc_hook()
    pname = nc.partition_id_tensor.name if nc.partition_id_tensor else None
    in_names, out_names, out_avals = [], [], []
    for alloc in nc.m.functions[0].allocations:
        if not isinstance(alloc, mybir.MemoryLocationSet):
            continue
        name = alloc.memorylocations[0].name
        if alloc.kind == "ExternalInput":
            if name != pname:
                in_names.append(name)
        elif alloc.kind == "ExternalOutput":
            out_names.append(name)
            out_avals.append(jax.core.ShapedArray(
                tuple(alloc.tensor_shape), mybir.dt.np(alloc.dtype)))
    n_params = len(in_names)
    all_names = in_names + out_names
    if pname is not None:
        all_names = all_names + [pname]

    def _body(*args):
        operands = list(args)
        if pname is not None:
            operands.append(partition_id_tensor())
        outs = _bass_exec_p.bind(
            *operands, out_avals=tuple(out_avals), in_names=tuple(all_names),
            out_names=tuple(out_names), lowering_input_output_aliases=(),
            sim_require_finite=True, sim_require_nnan=True, nc=nc)
        return tuple(outs)

    mesh = Mesh(np.asarray(jax.devices()[:N_CORES]), ("core",))
    nin = n_params + len(out_names)
    sharded = jax.jit(
        shard_map(_body, mesh=mesh,
                  in_specs=(PartitionSpec("core"),) * nin,
                  out_specs=(PartitionSpec("core"),) * len(out_names),
                  check_rep=False),
        keep_unused=True)
    zeros = [np.zeros((N_CORES * a.shape[0], *a.shape[1:]), a.dtype)
             for a in out_avals]
    _RUNNER_CACHE = (sharded, in_names, out_names, out_avals, zeros)
    return _RUNNER_CACHE


def _run_axon(in_maps):
    import jax
    sharded, in_names, out_names, out_avals, zeros = _axon_runner()
    concat_in = [
        np.concatenate([np.asarray(in_maps[c][n]) for c in range(N_CORES)],
                       axis=0)
        for n in in_names
    ]
    outs = sharded(*concat_in, *zeros)
    return [
        {n: np.asarray(outs[i]).reshape(N_CORES, *out_avals[i].shape)[c]
         for i, n in enumerate(out_names)}
        for c in range(N_CORES)
    ]


def run(inputs, **kw):
    """Returns (full_output, per-core results list)."""
    from concourse._compat import axon_active

    inputs = {k: np.asarray(v) for k, v in inputs.items()}
    in_maps = shard_inputs(**inputs)
    if axon_active():
        results = _run_axon(in_maps)
    else:
        results = run_bass_kernel_spmd(
            _get_nc(), in_maps, core_ids=list(range(N_CORES)), **kw).results
    B = 2
    out = np.zeros((B, S, D), np.float32)
    for c in range(N_CORES):
        b_idx = c // (N_CORES // 2)
        out[b_idx] += np.asarray(results[c]["outT"]).astype(np.float32).T
    return out, results


def kernel(**inputs):
    out, _ = run(inputs)
    return out


# revision 29
# speedup vs baseline: 1.0017x; 1.0017x over previous
"""Multi-head attention (B=2, S=2048, D=1024, H=16, dk=64) on 8 trn2 cores.

Sharding: batch (2) x head-group (4 heads each) = 8 shards.
Core c handles batch b = c // 4, heads g = c % 4 (heads 4g..4g+3).

Host-side prep per core (unchanged from v1):
  - inputs transposed to [d, s] so the contraction dim lands on SBUF
    partitions with no on-chip transposes,
  - Wq/Wk/Wv column-sharded per head group (1/sqrt(dk) folded into Wq/bq),
  - Wo row-sharded, transposed to [c, j],
  - each core emits a transposed partial output [1024, 2048]; host sums the
    4 partials per batch (bo/4 folded into each partial) and transposes back.

On-chip dataflow v2 (per core) - differences vs v1:
  - attn@V runs with au (=exp scores) as the STATIONARY operand and V as the
    moving operand: out[q, dv'] accumulates over k-tiles.  Matmul cost on the
    PE is (moving columns) per instruction, so this costs 65 cols per k-tile
    instead of 512, cutting attn@V PE time in half vs v1.
  - the denominator lands in column 64 of the same [q, 65] PSUM tile (ones
    column in V'), so softmax normalization is a per-partition reciprocal +
    tensor_scalar multiply on the DVE - the v1 PE-broadcast contraption
    (K=1 matmuls of a hi/lo bf16 split) is gone.
  - attention output is produced in [q, c] layout; a cheap PE transpose
    (identity matmul) flips it to [c, q] for the output projection.
  - instruction emission interleaves projection / attn@V / transpose /
    out-proj matmuls between score-matmul rounds so the PE and Act engines
    (exp is ~133us of Act time) both stay busy; score PSUM rotates through
    2 double-buffered 2-slot tiles, attn@V and general work each get 2
    more PSUM banks (8 banks exactly).

All matmuls run in bf16 (fp32 PSUM accumulate).  Full-kernel relative error
vs the fp32 reference is ~6e-3.
"""

from collections import deque
from contextlib import ExitStack

import ml_dtypes
import numpy as np

import concourse.bacc as bacc
import concourse.mybir as mybir
import concourse.tile as tile
from concourse.bass_utils import run_bass_kernel_spmd

F32 = mybir.dt.float32
BF16 = mybir.dt.bfloat16

D = 1024          # d_model
S = 2048          # sequence length
HCORE = 4         # heads per core
DK = 64           # head dim
M = HCORE * DK    # 256 sharded projection width
P = 128

N_CORES = 8
ST = 512          # s-tile (free dim of projection / q-tile)
N_ST = S // ST    # 4
N_DT = D // P     # 8 contraction tiles for projections
N_KT = S // P     # 16 k-tiles for attention
N_JT = D // P     # 8 output row tiles
N_PAIR = N_ST * HCORE   # 16 (qt, h) pairs
N_RND = 8         # score rounds per pair, 2 k-tiles each


def build_mha_tile(tc, outs, ins):
    nc = tc.nc
    xqT, xkT, xvT = ins["xqT"], ins["xkT"], ins["xvT"]
    wq, wk, wv, woT = ins["wq"], ins["wk"], ins["wv"], ins["woT"]
    bq, bk, bvb, bo4 = ins["bq"], ins["bk"], ins["bvb"], ins["bo4"]
    ident = ins["ident"]
    outT = outs["outT"]

    ctx = ExitStack()
    ec = ctx.enter_context
    const = ec(tc.tile_pool(name="const", bufs=1))
    persist = ec(tc.tile_pool(name="persist", bufs=1))
    xin_pool = ec(tc.tile_pool(name="xin", bufs=9))
    au_pool = ec(tc.tile_pool(name="au", bufs=26))
    ob_pool = ec(tc.tile_pool(name="outb", bufs=2))
    small = ec(tc.tile_pool(name="small", bufs=4))
    sc_ps = ec(tc.tile_pool(name="sc_ps", bufs=2, space="PSUM"))
    av_ps = ec(tc.tile_pool(name="av_ps", bufs=2, space="PSUM"))
    pp_ps = ec(tc.tile_pool(name="pp_ps", bufs=2, space="PSUM"))

    xqT3 = xqT.rearrange("(dt p) s -> p dt s", p=P)
    xkT3 = xkT.rearrange("(dt p) s -> p dt s", p=P)
    xvT3 = xvT.rearrange("(dt p) s -> p dt s", p=P)
    outT3 = outT.rearrange("(jt p) s -> p jt s", p=P)

    # ---- persistent activations ----
    QT_sb = persist.tile([P, 2, S], BF16)          # [p, mt, s]
    KT_sb = persist.tile([P, 2, S], BF16)
    V_sb = persist.tile([P, N_KT, HCORE, DK + 1], BF16)   # [p, kt, h, dv']
    cat_sb = persist.tile([P, N_KT, M], BF16)      # attn out [q-part, qtile, c]
    catT_sb = persist.tile([P, 2, S], BF16)        # transposed [c-part, ct, q]

    nc.vector.memset(V_sb[:, :, :, DK], 1.0)      # ones column for denominators

    # PE p-state warmup: ~3us of dummy matmuls on a memset tile so the ramp
    # to full clock is over before the first projection data lands
    spin = persist.tile([P, ST], BF16, name="spin")
    nc.gpsimd.memset(spin, 0.0)
    for i in range(8):
        ps = pp_ps.tile([P, ST], F32, tag="pp", name="spin_ps")
        nc.tensor.matmul(ps, spin[:, 0:P], spin, start=True, stop=True)

    # The `warm` DVE touches absorb each bias tile's DMA-lane wait so it
    # never lands as a 2nd sync wait on a hot DVE op (walrus allows only 1).
    warm = const.tile([P, 16], F32)

    # ---- const + input stream DMAs over three parallel issue queues (SP,
    # Activation, gpsimd/SWDGE) so the K st0 / Q st0 projections that gate
    # the first exp are not serialized behind each other's transfers.
    # Weights are loaded in mt-halves: only the mt0 halves gate pair 0. ----
    xin = {}

    def load_x(eng, key, src3, st, split=False, interleave_with=None):
        t = xin_pool.tile([P, N_DT, ST], BF16, tag="xt", name="xt")
        if split:
            # two half transfers so the first projection granule can start
            # after only 0.5MB has landed; optionally interleave another
            # split load between the halves (xk0a, xq0a, xk0b, xq0b)
            h = ST // 2
            eng.dma_start(t[:, :, 0:h], src3[:, :, st * ST:st * ST + h])
            xin[(key, st)] = t
            if interleave_with is not None:
                interleave_with()
            eng.dma_start(t[:, :, h:ST], src3[:, :, st * ST + h:(st + 1) * ST])
        else:
            eng.dma_start(t, src3[:, :, st * ST:(st + 1) * ST])
            xin[(key, st)] = t

    # All transfers serialize on the one modeled DMA device (~2.9us/MB), so
    # a single SP-queue stream in exact need order IS the transfer schedule:
    # Q/K st0 halves first (gate the first exp), then K st1-3 (score rounds
    # of pair 0), the mt1 weight halves (pair 2), V path, then out-proj
    # constants.  xq st1-3 ride the gpsimd queue later (they reuse xk bufs).
    wk_sb = [const.tile([P, N_DT, P], BF16, name=f"wk{m}_sb") for m in range(2)]
    wq_sb = [const.tile([P, N_DT, P], BF16, name=f"wq{m}_sb") for m in range(2)]
    nc.sync.dma_start(wk_sb[0], wk[0])
    nc.sync.dma_start(wq_sb[0], wq[0])
    load_x(nc.sync, "k", xkT3, 0, split=True, interleave_with=lambda: load_x(
        nc.sync, "q", xqT3, 0, split=True))
    bk_sb = const.tile([P, 2], F32, name="bk_sb")
    nc.sync.dma_start(bk_sb, bk)
    nc.vector.tensor_copy(warm[:, 2:4], bk_sb)
    bq_sb = const.tile([P, 2], F32, name="bq_sb")
    nc.sync.dma_start(bq_sb, bq)
    nc.vector.tensor_copy(warm[:, 0:2], bq_sb)
    for st in range(1, N_ST):
        load_x(nc.sync, "k", xkT3, st)
    wv_sb = const.tile([P, N_DT, M], BF16, name="wv_sb")
    nc.sync.dma_start(wv_sb, wv)
    bvb_sb = const.tile([P, M], F32, name="bvb_sb")
    nc.sync.dma_start(bvb_sb, bvb)
    nc.vector.tensor_copy(warm[:, 12:16], bvb_sb[:, 0:4])
    nc.sync.dma_start(wk_sb[1], wk[1])
    nc.sync.dma_start(wq_sb[1], wq[1])
    for st in range(N_ST):
        load_x(nc.sync, "v", xvT3, st)
    id_sb = const.tile([P, P], BF16, name="id_sb")
    nc.sync.dma_start(id_sb, ident)
    bo4_sb = const.tile([P, N_JT], F32, name="bo4_sb")
    nc.sync.dma_start(bo4_sb, bo4)
    nc.vector.tensor_copy(warm[:, 4:12], bo4_sb)
    woT_sb = const.tile([P, 2, D], BF16, name="woT_sb")
    nc.sync.dma_start(woT_sb, woT)

    # ---- emission helpers ----
    HS = ST // 2      # half-ST projection granule (2048 PE cycles)

    def proj_qk_mt(key, w_sb, b_sb, dst_sb, st, mt, half):
        """dst[:, mt, half-st-slice] = w.T @ xT + b, one 2048-cycle granule."""
        xt = xin[(key, st)]
        hs = slice(half * HS, (half + 1) * HS)
        ps = pp_ps.tile([P, ST], F32, tag="pp", name="qk_ps")[:, :HS]
        for dt in range(N_DT):
            nc.tensor.matmul(
                ps,
                w_sb[mt][:, dt, :],
                xt[:, dt, hs],
                start=(dt == 0), stop=(dt == N_DT - 1))
        nc.vector.tensor_scalar_add(
            dst_sb[:, mt, st * ST + half * HS:st * ST + (half + 1) * HS],
            ps, b_sb[:, mt:mt + 1])

    def v_chunk(h, ktg):
        """V[kt-rows, h, dv] for k-tiles 4*ktg..4*ktg+3, one head only, so
        attn@V of head h only ever waits for its own quarter of V-proj."""
        xt = xin[("v", ktg)]
        for kl in range(4):
            kt = 4 * ktg + kl
            ps = pp_ps.tile([P, ST], F32, tag="pp", name="v_ps")[:, :DK]
            for dt in range(N_DT):
                nc.tensor.matmul(
                    ps, xt[:, dt, kl * P:(kl + 1) * P],
                    wv_sb[:, dt, h * DK:(h + 1) * DK],
                    start=(dt == 0), stop=(dt == N_DT - 1))
            nc.vector.tensor_add(
                out=V_sb[:, kt, h, 0:DK],
                in0=ps, in1=bvb_sb[:, h * DK:(h + 1) * DK])

    def transp_chunk(qt, ct):
        """catT[:, ct, qt-block] = cat[qt-block, ct-block].T via PE."""
        for qtg in range(4):
            qq = qt * 4 + qtg
            tp = pp_ps.tile([P, 1024], BF16, tag="pp", name="tp_ps")[:, :P]
            nc.tensor.transpose(
                tp, cat_sb[:, qq, ct * P:(ct + 1) * P], id_sb)
            nc.vector.tensor_copy(
                catT_sb[:, ct, qq * P:(qq + 1) * P], tp)

    ob_tiles = {}

    def outp_chunk(qt, jt):
        """outT[jt, qt-slice] = woT.T @ catT + bo/4, DMA at jt 3 and 7."""
        qs = slice(qt * ST, (qt + 1) * ST)
        if jt == 0:
            ob_tiles[qt] = ob_pool.tile([P, N_JT, ST], BF16, tag="ob",
                                        name="ob")
        ob = ob_tiles[qt]
        ps = pp_ps.tile([P, ST], F32, tag="pp", name="op_ps")
        for ct in range(2):
            nc.tensor.matmul(
                ps,
                woT_sb[:, ct, jt * P:(jt + 1) * P],
                catT_sb[:, ct, qs],
                start=(ct == 0), stop=(ct == 1))
        if qt == N_ST - 1 and jt % 2 == 0:
            # the exp stream is over by now: the idle Act engine takes half
            # the bias-adds so the tail is PE-bound instead of DVE-bound
            nc.scalar.activation(
                ob[:, jt, :], ps, mybir.ActivationFunctionType.Identity,
                bias=bo4_sb[:, jt:jt + 1])
        else:
            nc.vector.tensor_scalar_add(ob[:, jt, :], ps, bo4_sb[:, jt:jt + 1])
        if qt == N_ST - 1:
            # final tile: fine-grained DMA so the drain only waits on the
            # last 2-jt slab instead of a 4-jt half
            if jt % 2 == 1:
                nc.sync.dma_start(outT3[:, jt - 1:jt + 1, qs],
                                  ob[:, jt - 1:jt + 1, :])
        elif jt == N_JT // 2 - 1:
            nc.sync.dma_start(outT3[:, :N_JT // 2, qs], ob[:, :N_JT // 2, :])
        elif jt == N_JT - 1:
            nc.sync.dma_start(outT3[:, N_JT // 2:, qs], ob[:, N_JT // 2:, :])

    # ---- filler scheduler ----
    round_clock = [0]      # sc rounds emitted so far (proxy for Act time)
    # xv chunk q becomes usable at ~(24.6 + 2.9q)us on the serial DMA device;
    # in Act rounds (first exp ~13us, 1.04us/round) that is round ~11+3q.
    VREADY = [13, 16, 19, 22]
    fillers = deque()
    remaining = {}

    def enq(key, cycles, fn):
        fillers.append((key, cycles, fn))
        remaining[key] = remaining.get(key, 0) + 1

    def run_next():
        key, cy, fn = fillers.popleft()
        fn()
        remaining[key] -= 1
        return cy

    def v_gated(key):
        return (key[0] == "V" and round_clock[0] < VREADY[key[2]])

    def fill(budget):
        # pop filler granules, skipping V-proj chunks whose xvT slab is
        # still in flight on the serial DMA device (they would head-of-line
        # block the PE); V chunks have no ordering ties to other entries
        while fillers and budget > 0:
            for i, (key, cy, fn) in enumerate(fillers):
                if not v_gated(key):
                    fillers.rotate(-i)
                    budget -= run_next()
                    fillers.rotate(i)
                    break
            else:
                return

    def drain_until(key):
        while remaining.get(key, 0) > 0:
            run_next()

    def enq_k(st, mt):
        for half in range(2):
            enq(("K", st, mt), 2048,
                lambda half=half: proj_qk_mt("k", wk_sb, bk_sb, KT_sb,
                                             st, mt, half))

    def enq_q(st, mt):
        for half in range(2):
            enq(("Q", st, mt), 2048,
                lambda half=half: proj_qk_mt("q", wq_sb, bq_sb, QT_sb,
                                             st, mt, half))

    def enq_v(h):
        for ktg in range(4):
            enq(("V", h, ktg), 2048, lambda h=h, ktg=ktg: v_chunk(h, ktg))

    def enq_xq(st):
        # late xq loads reuse xk bufs; emitted only after the matching
        # ("K", st-1, 1) filler so the WAR on the old tile is tracked
        enq(("D", st), 0, lambda st=st: load_x(nc.gpsimd, "q", xqT3, st))

    # mt0 chunks feed heads 0/1 (pairs 0-1), mt1 chunks heads 2/3 (pairs 2-3)
    for st in range(1, N_ST):
        enq_k(st, 0)
    enq_q(0, 1)
    enq_k(0, 1)
    enq_k(1, 1)
    enq_xq(1)
    enq_k(2, 1)
    enq_xq(2)
    enq_k(3, 1)
    enq_xq(3)
    enq_v(0)
    enq_v(1)
    enq_q(1, 0)
    enq_q(1, 1)
    enq_v(2)
    enq_q(2, 0)
    enq_q(2, 1)
    enq_v(3)
    enq_q(3, 0)
    enq_q(3, 1)

    # ---- attention emission ----
    def sc_round(qt, h, r):
        """Scores for k-tiles (2r, 2r+1) -> exp -> au tile [P, 2, ST]."""
        mt, p0 = h // 2, (h % 2) * DK
        qs = slice(qt * ST, (qt + 1) * ST)
        round_clock[0] += 1
        sc = sc_ps.tile([P, 2, ST], F32, tag="sc")
        au = au_pool.tile([P, 2, ST], BF16, tag="au")
        for i in range(2):
            kt = 2 * r + i
            nc.tensor.matmul(
                sc[:, i, :],
                KT_sb[p0:p0 + DK, mt, kt * P:(kt + 1) * P],
                QT_sb[p0:p0 + DK, mt, qs],
                start=True, stop=True)
        nc.scalar.activation(au, sc, mybir.ActivationFunctionType.Exp)
        return au

    # attn@V runs off a quarter-granular work queue: each item is 4 k-tiles
    # of one (pair, qtile) accumulation chain.  The queue self-paces against
    # V-projection availability (DMA-ordered) and naturally pipelines the
    # final pair's chains against its own exp stream.
    avq = deque()
    av_open = {}
    chains_done = {}

    def avq_append_pair(p, qt, h, au_tiles, qtgs=(0, 1, 2, 3),
                        interleave=False):
        if interleave:
            for quarter in range(4):
                for qtg in qtgs:
                    avq.append((qt, h, au_tiles, qtg, quarter))
        else:
            for qtg in qtgs:
                for quarter in range(4):
                    avq.append((qt, h, au_tiles, qtg, quarter))

    def avq_step(n):
        """Emit up to n attn@V quarters (~260 cycles each)."""
        done = 0
        while avq and done < n:
            qt, h, au_tiles, qtg, quarter = avq[0]
            if len(au_tiles) < 2 * (quarter + 1):
                break          # exp for these k-tiles not emitted yet
            if round_clock[0] < VREADY[quarter]:
                break          # V-proj input for these k-tiles still in DMA
            avq.popleft()
            drain_until(("V", h, quarter))
            if quarter == 0:
                ps = av_ps.tile([P, ST], F32, tag="av", name="av_ps")[:, :DK + 1]
                av_open[(qt, h, qtg)] = ps
            else:
                ps = av_open[(qt, h, qtg)]
            q0 = qtg * P
            for kl in range(4):
                kt = 4 * quarter + kl
                nc.tensor.matmul(
                    ps,
                    au_tiles[kt // 2][:, kt % 2, q0:q0 + P],
                    V_sb[:, kt, h, :],
                    start=(kt == 0), stop=(kt == N_KT - 1))
            if quarter == 3:
                av_open.pop((qt, h, qtg))
                recip = small.tile([P, 1], F32, tag="recip")
                nc.vector.reciprocal(recip, ps[:, DK:DK + 1])
                nc.vector.tensor_scalar_mul(
                    cat_sb[:, qt * 4 + qtg, h * DK:(h + 1) * DK],
                    ps[:, 0:DK], recip)
                nd = chains_done[(qt, h)] = chains_done.get((qt, h), 0) + 1
                if nd == 4:
                    # this pair's attention output slab is complete: its
                    # transpose (and out-proj once all heads are in) can go
                    if h == 1:
                        enq(("T", qt, 0), 512,
                            lambda qt=qt: transp_chunk(qt, 0))
                    elif h == 3:
                        enq(("T", qt, 1), 512,
                            lambda qt=qt: transp_chunk(qt, 1))
                        for jt in range(N_JT):
                            enq(("O", qt, jt), 1024,
                                lambda qt=qt, jt=jt: outp_chunk(qt, jt))
            done += 1

    # pre-pair work: only the mt0 halves of K st0 / Q st0 gate pair 0,
    # emitted in DMA-arrival order (xk0a, xq0a, xq0b, xk0b)
    proj_qk_mt("k", wk_sb, bk_sb, KT_sb, 0, 0, 0)
    proj_qk_mt("q", wq_sb, bq_sb, QT_sb, 0, 0, 0)
    proj_qk_mt("q", wq_sb, bq_sb, QT_sb, 0, 0, 1)
    proj_qk_mt("k", wk_sb, bk_sb, KT_sb, 0, 0, 1)

    for p in range(N_PAIR):
        qt, h = divmod(p, HCORE)
        mt = h // 2
        drain_until(("Q", qt, mt))
        last = p == N_PAIR - 1
        au_tiles = []
        for r in range(N_RND):
            drain_until(("K", r // 2, mt))
            au_tiles.append(sc_round(qt, h, r))
            # lookahead drains: pull future deps in small steps so pair/round
            # boundaries never force a large burst of queued filler work
            if p == 1 and r % 2 == 0:
                drain_until(("K", r // 2, 1))
            if h == 1 and r == 4:
                drain_until(("Q", qt, 1))
            if h == 3 and r == 4 and qt < 3:
                drain_until(("Q", qt + 1, 0))
            if last and r == 1:
                # feed this pair's first two chains as its own exps land
                avq_append_pair(p, qt, h, au_tiles, qtgs=(0, 1),
                                interleave=True)
            backlog = len(avq)
            avq_step(4 if last else (3 if backlog > 20 else 2))
            fill(1300 if p >= 13 else (2000 if p == 0 else 900))
        if not last:
            avq_append_pair(p, qt, h, au_tiles)
        else:
            avq_append_pair(p, qt, h, au_tiles, qtgs=(2, 3))

    # ---- tail: drain remaining attn@V quarters, then final out-proj ----
    while avq:
        avq_step(4)
        fill(1000)
    while fillers:
        run_next()
    ctx.close()


def build_bass():
    nc = bacc.Bacc(trn_type="TRN2", target_bir_lowering=False, debug=False)
    ins = {
        "xqT": nc.dram_tensor("xqT", (D, S), BF16, kind="ExternalInput").ap(),
        "xkT": nc.dram_tensor("xkT", (D, S), BF16, kind="ExternalInput").ap(),
        "xvT": nc.dram_tensor("xvT", (D, S), BF16, kind="ExternalInput").ap(),
        "wq": nc.dram_tensor("wq", (2, P, N_DT, P), BF16, kind="ExternalInput").ap(),
        "wk": nc.dram_tensor("wk", (2, P, N_DT, P), BF16, kind="ExternalInput").ap(),
        "wv": nc.dram_tensor("wv", (P, N_DT, M), BF16, kind="ExternalInput").ap(),
        "woT": nc.dram_tensor("woT", (P, 2, D), BF16, kind="ExternalInput").ap(),
        "bq": nc.dram_tensor("bq", (P, 2), F32, kind="ExternalInput").ap(),
        "bk": nc.dram_tensor("bk", (P, 2), F32, kind="ExternalInput").ap(),
        "bvb": nc.dram_tensor("bvb", (P, M), F32, kind="ExternalInput").ap(),
        "bo4": nc.dram_tensor("bo4", (P, N_JT), F32, kind="ExternalInput").ap(),
        "ident": nc.dram_tensor("ident", (P, P), BF16, kind="ExternalInput").ap(),
    }
    outs = {
        "outT": nc.dram_tensor("outT", (D, S), BF16, kind="ExternalOutput").ap(),
    }
    with tile.TileContext(nc) as tc:
        build_mha_tile(tc, outs, ins)
    nc.compile()
    return nc


def shard_inputs(query, key, value, Wq, bq, Wk, bk, Wv, bv, Wo, bo):
    """Build the 8 per-core input maps (all host-side numpy layout prep)."""
    def prep_w(W, ms, scale=1.0, mt_major=False):
        # [d, m] -> [p, dt, m]  (or mt-major [2, p, dt, 128] for split loads)
        wT = (np.asarray(W)[ms, :].T * scale).astype(ml_dtypes.bfloat16)
        w = wT.reshape(N_DT, P, M).transpose(1, 0, 2)
        if mt_major:
            w = np.stack([w[:, :, 0:P], w[:, :, P:M]])
        return np.ascontiguousarray(w)

    def prep_b(b, ms, scale=1.0):
        return np.ascontiguousarray(
            (np.asarray(b)[ms] * scale).astype(np.float32).reshape(2, P).T)

    ident = np.eye(P, dtype=ml_dtypes.bfloat16)
    in_maps = []
    for c in range(N_CORES):
        b_idx, g = divmod(c, N_CORES // 2)
        ms = slice(g * M, (g + 1) * M)
        woT = np.ascontiguousarray(Wo[:, ms].T.astype(np.float32))
        in_maps.append({
            "xqT": np.ascontiguousarray(query[b_idx].T.astype(ml_dtypes.bfloat16)),
            "xkT": np.ascontiguousarray(key[b_idx].T.astype(ml_dtypes.bfloat16)),
            "xvT": np.ascontiguousarray(value[b_idx].T.astype(ml_dtypes.bfloat16)),
            "wq": prep_w(Wq, ms, 1.0 / np.sqrt(DK), mt_major=True),
            "wk": prep_w(Wk, ms, mt_major=True),
            "wv": prep_w(Wv, ms),
            "woT": np.ascontiguousarray(
                woT.astype(ml_dtypes.bfloat16).reshape(2, P, D).transpose(1, 0, 2)),
            "bq": prep_b(bq, ms, 1.0 / np.sqrt(DK)),
            "bk": prep_b(bk, ms),
            "bvb": np.ascontiguousarray(
                np.tile(np.asarray(bv)[ms].astype(np.float32), (P, 1))),
            "bo4": np.ascontiguousarray(
                (np.asarray(bo) / (N_CORES // 2)).astype(np.float32)
                .reshape(N_JT, P).T),
            "ident": ident,
        })
    return in_maps


_NC_CACHE = None
_RUNNER_CACHE = None


def _get_nc():
    global _NC_CACHE
    if _NC_CACHE is None:
        _NC_CACHE = build_bass()
    return _NC_CACHE


def _axon_runner():
    """Jit the SPMD NEFF exec once (no donation; kernel writes every output
    element, so reusing non-donated zero buffers across calls is safe)."""
    global _RUNNER_CACHE
    if _RUNNER_CACHE is not None:
        return _RUNNER_CACHE
    import jax
    from jax.experimental.shard_map import shard_map
    from jax.sharding import Mesh, PartitionSpec
    from concourse.bass2jax import (_bass_exec_p, install_neuronx_cc_hook,
                                    partition_id_tensor)

    nc = _get_nc()
    install_neuronx_cc_hook()
    pname = nc.partition_id_tensor.name if nc.partition_id_tensor else None
    in_names, out_names, out_avals = [], [], []
    for alloc in nc.m.functions[0].allocations:
        if not isinstance(alloc, mybir.MemoryLocationSet):
            continue
        name = alloc.memorylocations[0].name
        if alloc.kind == "ExternalInput":
            if name != pname:
                in_names.append(name)
        elif alloc.kind == "ExternalOutput":
            out_names.append(name)
            out_avals.append(jax.core.ShapedArray(
                tuple(alloc.tensor_shape), mybir.dt.np(alloc.dtype)))
    n_params = len(in_names)
    all_names = in_names + out_names
    if pname is not None:
        all_names = all_names + [pname]

    def _body(*args):
        operands = list(args)
        if pname is not None:
            operands.append(partition_id_tensor())
        outs = _bass_exec_p.bind(
            *operands, out_avals=tuple(out_avals), in_names=tuple(all_names),
            out_names=tuple(out_names), lowering_input_output_aliases=(),
            sim_require_finite=True, sim_require_nnan=True, nc=nc)
        return tuple(outs)

    mesh = Mesh(np.asarray(jax.devices()[:N_CORES]), ("core",))
    nin = n_params + len(out_names)
    sharded = jax.jit(
        shard_map(_body, mesh=mesh,
                  in_specs=(PartitionSpec("core"),) * nin,
                  out_specs=(PartitionSpec("core"),) * len(out_names),
                  check_rep=False),
        keep_unused=True)
    zeros = [np.zeros((N_CORES * a.shape[0], *a.shape[1:]), a.dtype)
             for a in out_avals]
    _RUNNER_CACHE = (sharded, in_names, out_names, out_avals, zeros)
    return _RUNNER_CACHE


def _run_axon(in_maps):
    import jax
    sharded, in_names, out_names, out_avals, zeros = _axon_runner()
    concat_in = [
        np.concatenate([np.asarray(in_maps[c][n]) for c in range(N_CORES)],
                       axis=0)
        for n in in_names
    ]
    outs = sharded(*concat_in, *zeros)
    return [
        {n: np.asarray(outs[i]).reshape(N_CORES, *out_avals[i].shape)[c]
         for i, n in enumerate(out_names)}
        for c in range(N_CORES)
    ]


def run(inputs, **kw):
    """Returns (full_output, per-core results list)."""
    from concourse._compat import axon_active

    inputs = {k: np.asarray(v) for k, v in inputs.items()}
    in_maps = shard_inputs(**inputs)
    if axon_active():
        results = _run_axon(in_maps)
    else:
        results = run_bass_kernel_spmd(
            _get_nc(), in_maps, core_ids=list(range(N_CORES)), **kw).results
    B = 2
    out = np.zeros((B, S, D), np.float32)
    for c in range(N_CORES):
        b_idx = c // (N_CORES // 2)
        out[b_idx] += np.asarray(results[c]["outT"]).astype(np.float32).T
    return out, results


def kernel(**inputs):
    out, _ = run(inputs)
    return out


# revision 30
# speedup vs baseline: 1.0184x; 1.0167x over previous
"""Multi-head attention (B=2, S=2048, D=1024, H=16, dk=64) on 8 trn2 cores.

Sharding: batch (2) x head-group (4 heads each) = 8 shards.
Core c handles batch b = c // 4, heads g = c % 4 (heads 4g..4g+3).

Host-side prep per core (unchanged from v1):
  - inputs transposed to [d, s] so the contraction dim lands on SBUF
    partitions with no on-chip transposes,
  - Wq/Wk/Wv column-sharded per head group (1/sqrt(dk) folded into Wq/bq),
  - Wo row-sharded, transposed to [c, j],
  - each core emits a transposed partial output [1024, 2048]; host sums the
    4 partials per batch (bo/4 folded into each partial) and transposes back.

On-chip dataflow v2 (per core) - differences vs v1:
  - attn@V runs with au (=exp scores) as the STATIONARY operand and V as the
    moving operand: out[q, dv'] accumulates over k-tiles.  Matmul cost on the
    PE is (moving columns) per instruction, so this costs 65 cols per k-tile
    instead of 512, cutting attn@V PE time in half vs v1.
  - the denominator lands in column 64 of the same [q, 65] PSUM tile (ones
    column in V'), so softmax normalization is a per-partition reciprocal +
    tensor_scalar multiply on the DVE - the v1 PE-broadcast contraption
    (K=1 matmuls of a hi/lo bf16 split) is gone.
  - attention output is produced in [q, c] layout; a cheap PE transpose
    (identity matmul) flips it to [c, q] for the output projection.
  - instruction emission interleaves projection / attn@V / transpose /
    out-proj matmuls between score-matmul rounds so the PE and Act engines
    (exp is ~133us of Act time) both stay busy; score PSUM rotates through
    2 double-buffered 2-slot tiles, attn@V and general work each get 2
    more PSUM banks (8 banks exactly).

All matmuls run in bf16 (fp32 PSUM accumulate).  Full-kernel relative error
vs the fp32 reference is ~6e-3.
"""

from collections import deque
from contextlib import ExitStack

import ml_dtypes
import numpy as np

import concourse.bacc as bacc
import concourse.mybir as mybir
import concourse.tile as tile
from concourse.bass_utils import run_bass_kernel_spmd

F32 = mybir.dt.float32
BF16 = mybir.dt.bfloat16

D = 1024          # d_model
S = 2048          # sequence length
HCORE = 4         # heads per core
DK = 64           # head dim
M = HCORE * DK    # 256 sharded projection width
P = 128

N_CORES = 8
ST = 512          # s-tile (free dim of projection / q-tile)
N_ST = S // ST    # 4
N_DT = D // P     # 8 contraction tiles for projections
N_KT = S // P     # 16 k-tiles for attention
N_JT = D // P     # 8 output row tiles
N_PAIR = N_ST * HCORE   # 16 (qt, h) pairs
N_RND = 8         # score rounds per pair, 2 k-tiles each


def build_mha_tile(tc, outs, ins):
    nc = tc.nc
    xqT, xkT, xvT = ins["xqT"], ins["xkT"], ins["xvT"]
    wq, wk, wv, woT = ins["wq"], ins["wk"], ins["wv"], ins["woT"]
    bq, bk, bvb, bo4 = ins["bq"], ins["bk"], ins["bvb"], ins["bo4"]
    ident = ins["ident"]
    outT = outs["outT"]

    ctx = ExitStack()
    ec = ctx.enter_context
    const = ec(tc.tile_pool(name="const", bufs=1))
    persist = ec(tc.tile_pool(name="persist", bufs=1))
    xin_pool = ec(tc.tile_pool(name="xin", bufs=9))
    au_pool = ec(tc.tile_pool(name="au", bufs=26))
    ob_pool = ec(tc.tile_pool(name="outb", bufs=2))
    small = ec(tc.tile_pool(name="small", bufs=4))
    sc_ps = ec(tc.tile_pool(name="sc_ps", bufs=2, space="PSUM"))
    av_ps = ec(tc.tile_pool(name="av_ps", bufs=2, space="PSUM"))
    pp_ps = ec(tc.tile_pool(name="pp_ps", bufs=2, space="PSUM"))

    xqT3 = xqT.rearrange("(dt p) s -> p dt s", p=P)
    xkT3 = xkT.rearrange("(dt p) s -> p dt s", p=P)
    xvT3 = xvT.rearrange("(dt p) s -> p dt s", p=P)
    outT3 = outT.rearrange("(jt p) s -> p jt s", p=P)

    # ---- persistent activations ----
    QT_sb = persist.tile([P, 2, S], BF16)          # [p, mt, s]
    KT_sb = persist.tile([P, 2, S], BF16)
    V_sb = persist.tile([P, N_KT, HCORE, DK + 1], BF16)   # [p, kt, h, dv']
    cat_sb = persist.tile([P, N_KT, M], BF16)      # attn out [q-part, qtile, c]
    catT_sb = persist.tile([P, 2, S], BF16)        # transposed [c-part, ct, q]

    nc.vector.memset(V_sb[:, :, :, DK], 1.0)      # ones column for denominators

    # PE p-state warmup: ~3us of dummy matmuls on a memset tile so the ramp
    # to full clock is over before the first projection data lands
    spin = persist.tile([P, ST], BF16, name="spin")
    nc.gpsimd.memset(spin, 0.0)
    for i in range(8):
        ps = pp_ps.tile([P, ST], F32, tag="pp", name="spin_ps")
        nc.tensor.matmul(ps, spin[:, 0:P], spin, start=True, stop=True)

    # The `warm` DVE touches absorb each bias tile's DMA-lane wait so it
    # never lands as a 2nd sync wait on a hot DVE op (walrus allows only 1).
    warm = const.tile([P, 16], F32)

    # ---- const + input stream DMAs over three parallel issue queues (SP,
    # Activation, gpsimd/SWDGE) so the K st0 / Q st0 projections that gate
    # the first exp are not serialized behind each other's transfers.
    # Weights are loaded in mt-halves: only the mt0 halves gate pair 0. ----
    xin = {}

    def load_x(eng, key, src3, st, split=False, interleave_with=None):
        t = xin_pool.tile([P, N_DT, ST], BF16, tag="xt", name="xt")
        if split:
            # two half transfers so the first projection granule can start
            # after only 0.5MB has landed; optionally interleave another
            # split load between the halves (xk0a, xq0a, xk0b, xq0b)
            h = ST // 2
            eng.dma_start(t[:, :, 0:h], src3[:, :, st * ST:st * ST + h])
            xin[(key, st)] = t
            if interleave_with is not None:
                interleave_with()
            eng.dma_start(t[:, :, h:ST], src3[:, :, st * ST + h:(st + 1) * ST])
        else:
            eng.dma_start(t, src3[:, :, st * ST:(st + 1) * ST])
            xin[(key, st)] = t

    # All transfers serialize on the one modeled DMA device (~2.9us/MB), so
    # a single SP-queue stream in exact need order IS the transfer schedule:
    # Q/K st0 halves first (gate the first exp), then K st1-3 (score rounds
    # of pair 0), the mt1 weight halves (pair 2), V path, then out-proj
    # constants.  xq st1-3 ride the gpsimd queue later (they reuse xk bufs).
    wk_sb = [const.tile([P, N_DT, P], BF16, name=f"wk{m}_sb") for m in range(2)]
    wq_sb = [const.tile([P, N_DT, P], BF16, name=f"wq{m}_sb") for m in range(2)]
    nc.sync.dma_start(wk_sb[0], wk[0])
    nc.sync.dma_start(wq_sb[0], wq[0])
    load_x(nc.sync, "k", xkT3, 0, split=True, interleave_with=lambda: load_x(
        nc.sync, "q", xqT3, 0, split=True))
    bk_sb = const.tile([P, 2], F32, name="bk_sb")
    nc.sync.dma_start(bk_sb, bk)
    nc.vector.tensor_copy(warm[:, 2:4], bk_sb)
    bq_sb = const.tile([P, 2], F32, name="bq_sb")
    nc.sync.dma_start(bq_sb, bq)
    nc.vector.tensor_copy(warm[:, 0:2], bq_sb)
    for st in range(1, N_ST):
        load_x(nc.sync, "k", xkT3, st)
    wv_sb = const.tile([P, N_DT, M], BF16, name="wv_sb")
    nc.sync.dma_start(wv_sb, wv)
    bvb_sb = const.tile([P, M], F32, name="bvb_sb")
    nc.sync.dma_start(bvb_sb, bvb)
    nc.vector.tensor_copy(warm[:, 12:16], bvb_sb[:, 0:4])
    nc.sync.dma_start(wk_sb[1], wk[1])
    nc.sync.dma_start(wq_sb[1], wq[1])
    for st in range(N_ST):
        load_x(nc.sync, "v", xvT3, st)
    id_sb = const.tile([P, P], BF16, name="id_sb")
    nc.sync.dma_start(id_sb, ident)
    bo4_sb = const.tile([P, N_JT], F32, name="bo4_sb")
    nc.sync.dma_start(bo4_sb, bo4)
    nc.vector.tensor_copy(warm[:, 4:12], bo4_sb)
    woT_sb = const.tile([P, 2, D], BF16, name="woT_sb")
    nc.sync.dma_start(woT_sb, woT)

    # ---- emission helpers ----
    HS = ST // 2      # half-ST projection granule (2048 PE cycles)

    def proj_qk_mt(key, w_sb, b_sb, dst_sb, st, mt, half):
        """dst[:, mt, half-st-slice] = w.T @ xT + b, one 2048-cycle granule."""
        xt = xin[(key, st)]
        hs = slice(half * HS, (half + 1) * HS)
        ps = pp_ps.tile([P, ST], F32, tag="pp", name="qk_ps")[:, :HS]
        for dt in range(N_DT):
            nc.tensor.matmul(
                ps,
                w_sb[mt][:, dt, :],
                xt[:, dt, hs],
                start=(dt == 0), stop=(dt == N_DT - 1))
        nc.vector.tensor_scalar_add(
            dst_sb[:, mt, st * ST + half * HS:st * ST + (half + 1) * HS],
            ps, b_sb[:, mt:mt + 1])

    def v_chunk(h, ktg):
        """V[kt-rows, h, dv] for k-tiles 4*ktg..4*ktg+3, one head only, so
        attn@V of head h only ever waits for its own quarter of V-proj."""
        xt = xin[("v", ktg)]
        for kl in range(4):
            kt = 4 * ktg + kl
            ps = pp_ps.tile([P, ST], F32, tag="pp", name="v_ps")[:, :DK]
            for dt in range(N_DT):
                nc.tensor.matmul(
                    ps, xt[:, dt, kl * P:(kl + 1) * P],
                    wv_sb[:, dt, h * DK:(h + 1) * DK],
                    start=(dt == 0), stop=(dt == N_DT - 1))
            nc.vector.tensor_add(
                out=V_sb[:, kt, h, 0:DK],
                in0=ps, in1=bvb_sb[:, h * DK:(h + 1) * DK])

    def transp_chunk(qt, ct):
        """catT[:, ct, qt-block] = cat[qt-block, ct-block].T via PE."""
        for qtg in range(4):
            qq = qt * 4 + qtg
            tp = pp_ps.tile([P, 1024], BF16, tag="pp", name="tp_ps")[:, :P]
            nc.tensor.transpose(
                tp, cat_sb[:, qq, ct * P:(ct + 1) * P], id_sb)
            nc.vector.tensor_copy(
                catT_sb[:, ct, qq * P:(qq + 1) * P], tp)

    ob_tiles = {}

    def outp_chunk(qt, jt):
        """outT[jt, qt-slice] = woT.T @ catT + bo/4, DMA at jt 3 and 7."""
        qs = slice(qt * ST, (qt + 1) * ST)
        if jt == 0:
            ob_tiles[qt] = ob_pool.tile([P, N_JT, ST], BF16, tag="ob",
                                        name="ob")
        ob = ob_tiles[qt]
        ps = pp_ps.tile([P, ST], F32, tag="pp", name="op_ps")
        for ct in range(2):
            nc.tensor.matmul(
                ps,
                woT_sb[:, ct, jt * P:(jt + 1) * P],
                catT_sb[:, ct, qs],
                start=(ct == 0), stop=(ct == 1))
        if qt == N_ST - 1 and jt % 2 == 0:
            # the exp stream is over by now: the idle Act engine takes half
            # the bias-adds so the tail is PE-bound instead of DVE-bound
            nc.scalar.activation(
                ob[:, jt, :], ps, mybir.ActivationFunctionType.Identity,
                bias=bo4_sb[:, jt:jt + 1])
        else:
            nc.vector.tensor_scalar_add(ob[:, jt, :], ps, bo4_sb[:, jt:jt + 1])
        if qt == N_ST - 1:
            # final tile: fine-grained DMA so the drain only waits on the
            # last 2-jt slab instead of a 4-jt half
            if jt % 2 == 1:
                nc.sync.dma_start(outT3[:, jt - 1:jt + 1, qs],
                                  ob[:, jt - 1:jt + 1, :])
        elif jt == N_JT // 2 - 1:
            nc.sync.dma_start(outT3[:, :N_JT // 2, qs], ob[:, :N_JT // 2, :])
        elif jt == N_JT - 1:
            nc.sync.dma_start(outT3[:, N_JT // 2:, qs], ob[:, N_JT // 2:, :])

    # ---- filler scheduler ----
    round_clock = [0]      # sc rounds emitted so far (proxy for Act time)
    # xv chunk q becomes usable at ~(24.6 + 2.9q)us on the serial DMA device;
    # in Act rounds (first exp ~13us, 1.04us/round) that is round ~11+3q.
    VREADY = [12, 15, 18, 21]
    fillers = deque()
    remaining = {}

    def enq(key, cycles, fn):
        fillers.append((key, cycles, fn))
        remaining[key] = remaining.get(key, 0) + 1

    def run_next():
        key, cy, fn = fillers.popleft()
        fn()
        remaining[key] -= 1
        return cy

    def fill(budget):
        while fillers and budget > 0:
            budget -= run_next()

    def drain_until(key):
        while remaining.get(key, 0) > 0:
            run_next()

    def enq_k(st, mt):
        for half in range(2):
            enq(("K", st, mt), 2048,
                lambda half=half: proj_qk_mt("k", wk_sb, bk_sb, KT_sb,
                                             st, mt, half))

    def enq_q(st, mt):
        for half in range(2):
            enq(("Q", st, mt), 2048,
                lambda half=half: proj_qk_mt("q", wq_sb, bq_sb, QT_sb,
                                             st, mt, half))

    def enq_v(h):
        for ktg in range(4):
            enq(("V", h, ktg), 2048, lambda h=h, ktg=ktg: v_chunk(h, ktg))

    def enq_xq(st):
        # late xq loads reuse xk bufs; emitted only after the matching
        # ("K", st-1, 1) filler so the WAR on the old tile is tracked
        enq(("D", st), 0, lambda st=st: load_x(nc.gpsimd, "q", xqT3, st))

    # mt0 chunks feed heads 0/1 (pairs 0-1), mt1 chunks heads 2/3 (pairs 2-3)
    for st in range(1, N_ST):
        enq_k(st, 0)
    enq_q(0, 1)
    enq_k(0, 1)
    enq_k(1, 1)
    enq_xq(1)
    enq_k(2, 1)
    enq_xq(2)
    enq_k(3, 1)
    enq_xq(3)
    enq_v(0)
    enq_v(1)
    enq_q(1, 0)
    enq_q(1, 1)
    enq_v(2)
    enq_q(2, 0)
    enq_q(2, 1)
    enq_v(3)
    enq_q(3, 0)
    enq_q(3, 1)

    # ---- attention emission ----
    def sc_round(qt, h, r):
        """Scores for k-tiles (2r, 2r+1) -> exp -> au tile [P, 2, ST]."""
        mt, p0 = h // 2, (h % 2) * DK
        qs = slice(qt * ST, (qt + 1) * ST)
        round_clock[0] += 1
        sc = sc_ps.tile([P, 2, ST], F32, tag="sc")
        au = au_pool.tile([P, 2, ST], BF16, tag="au")
        for i in range(2):
            kt = 2 * r + i
            nc.tensor.matmul(
                sc[:, i, :],
                KT_sb[p0:p0 + DK, mt, kt * P:(kt + 1) * P],
                QT_sb[p0:p0 + DK, mt, qs],
                start=True, stop=True)
        nc.scalar.activation(au, sc, mybir.ActivationFunctionType.Exp)
        return au

    # attn@V runs off a quarter-granular work queue: each item is 4 k-tiles
    # of one (pair, qtile) accumulation chain.  The queue self-paces against
    # V-projection availability (DMA-ordered) and naturally pipelines the
    # final pair's chains against its own exp stream.
    avq = deque()
    av_open = {}
    chains_done = {}

    def avq_append_pair(p, qt, h, au_tiles, qtgs=(0, 1, 2, 3),
                        interleave=False):
        if interleave:
            for quarter in range(4):
                for qtg in qtgs:
                    avq.append((qt, h, au_tiles, qtg, quarter))
        else:
            for qtg in qtgs:
                for quarter in range(4):
                    avq.append((qt, h, au_tiles, qtg, quarter))

    def avq_step(n):
        """Emit up to n attn@V quarters (~260 cycles each)."""
        done = 0
        while avq and done < n:
            qt, h, au_tiles, qtg, quarter = avq[0]
            if len(au_tiles) < 2 * (quarter + 1):
                break          # exp for these k-tiles not emitted yet
            if round_clock[0] < VREADY[quarter]:
                break          # V-proj input for these k-tiles still in DMA
            avq.popleft()
            drain_until(("V", h, quarter))
            if quarter == 0:
                ps = av_ps.tile([P, ST], F32, tag="av", name="av_ps")[:, :DK + 1]
                av_open[(qt, h, qtg)] = ps
            else:
                ps = av_open[(qt, h, qtg)]
            q0 = qtg * P
            for kl in range(4):
                kt = 4 * quarter + kl
                nc.tensor.matmul(
                    ps,
                    au_tiles[kt // 2][:, kt % 2, q0:q0 + P],
                    V_sb[:, kt, h, :],
                    start=(kt == 0), stop=(kt == N_KT - 1))
            if quarter == 3:
                av_open.pop((qt, h, qtg))
                recip = small.tile([P, 1], F32, tag="recip")
                nc.vector.reciprocal(recip, ps[:, DK:DK + 1])
                nc.vector.tensor_scalar_mul(
                    cat_sb[:, qt * 4 + qtg, h * DK:(h + 1) * DK],
                    ps[:, 0:DK], recip)
                nd = chains_done[(qt, h)] = chains_done.get((qt, h), 0) + 1
                if nd == 4:
                    # this pair's attention output slab is complete: its
                    # transpose (and out-proj once all heads are in) can go
                    if h == 1:
                        enq(("T", qt, 0), 512,
                            lambda qt=qt: transp_chunk(qt, 0))
                    elif h == 3:
                        enq(("T", qt, 1), 512,
                            lambda qt=qt: transp_chunk(qt, 1))
                        for jt in range(N_JT):
                            enq(("O", qt, jt), 1024,
                                lambda qt=qt, jt=jt: outp_chunk(qt, jt))
            done += 1

    # pre-pair work: only the mt0 halves of K st0 / Q st0 gate pair 0,
    # emitted in DMA-arrival order (xk0a, xq0a, xq0b, xk0b)
    proj_qk_mt("k", wk_sb, bk_sb, KT_sb, 0, 0, 0)
    proj_qk_mt("q", wq_sb, bq_sb, QT_sb, 0, 0, 0)
    proj_qk_mt("q", wq_sb, bq_sb, QT_sb, 0, 0, 1)
    proj_qk_mt("k", wk_sb, bk_sb, KT_sb, 0, 0, 1)

    for p in range(N_PAIR):
        qt, h = divmod(p, HCORE)
        mt = h // 2
        drain_until(("Q", qt, mt))
        last = p == N_PAIR - 1
        au_tiles = []
        for r in range(N_RND):
            drain_until(("K", r // 2, mt))
            au_tiles.append(sc_round(qt, h, r))
            # lookahead drains: pull future deps in small steps so pair/round
            # boundaries never force a large burst of queued filler work
            if p == 1 and r % 2 == 0:
                drain_until(("K", r // 2, 1))
            if h == 1 and r == 4:
                drain_until(("Q", qt, 1))
            if h == 3 and r == 4 and qt < 3:
                drain_until(("Q", qt + 1, 0))
            if last and r == 1:
                # feed this pair's first two chains as its own exps land
                avq_append_pair(p, qt, h, au_tiles, qtgs=(0, 1),
                                interleave=True)
            backlog = len(avq)
            avq_step(4 if last else (3 if backlog > 20 else 2))
            fill(1300 if p >= 13 else (2000 if p == 0 else 900))
        if not last:
            avq_append_pair(p, qt, h, au_tiles)
        else:
            avq_append_pair(p, qt, h, au_tiles, qtgs=(2, 3))

    # ---- tail: drain remaining attn@V quarters, then final out-proj ----
    while avq:
        avq_step(4)
        fill(1000)
    while fillers:
        run_next()
    ctx.close()


def build_bass():
    nc = bacc.Bacc(trn_type="TRN2", target_bir_lowering=False, debug=False)
    ins = {
        "xqT": nc.dram_tensor("xqT", (D, S), BF16, kind="ExternalInput").ap(),
        "xkT": nc.dram_tensor("xkT", (D, S), BF16, kind="ExternalInput").ap(),
        "xvT": nc.dram_tensor("xvT", (D, S), BF16, kind="ExternalInput").ap(),
        "wq": nc.dram_tensor("wq", (2, P, N_DT, P), BF16, kind="ExternalInput").ap(),
        "wk": nc.dram_tensor("wk", (2, P, N_DT, P), BF16, kind="ExternalInput").ap(),
        "wv": nc.dram_tensor("wv", (P, N_DT, M), BF16, kind="ExternalInput").ap(),
        "woT": nc.dram_tensor("woT", (P, 2, D), BF16, kind="ExternalInput").ap(),
        "bq": nc.dram_tensor("bq", (P, 2), F32, kind="ExternalInput").ap(),
        "bk": nc.dram_tensor("bk", (P, 2), F32, kind="ExternalInput").ap(),
        "bvb": nc.dram_tensor("bvb", (P, M), F32, kind="ExternalInput").ap(),
        "bo4": nc.dram_tensor("bo4", (P, N_JT), F32, kind="ExternalInput").ap(),
        "ident": nc.dram_tensor("ident", (P, P), BF16, kind="ExternalInput").ap(),
    }
    outs = {
        "outT": nc.dram_tensor("outT", (D, S), BF16, kind="ExternalOutput").ap(),
    }
    with tile.TileContext(nc) as tc:
        build_mha_tile(tc, outs, ins)
    nc.compile()
    return nc


def shard_inputs(query, key, value, Wq, bq, Wk, bk, Wv, bv, Wo, bo):
    """Build the 8 per-core input maps (all host-side numpy layout prep)."""
    def prep_w(W, ms, scale=1.0, mt_major=False):
        # [d, m] -> [p, dt, m]  (or mt-major [2, p, dt, 128] for split loads)
        wT = (np.asarray(W)[ms, :].T * scale).astype(ml_dtypes.bfloat16)
        w = wT.reshape(N_DT, P, M).transpose(1, 0, 2)
        if mt_major:
            w = np.stack([w[:, :, 0:P], w[:, :, P:M]])
        return np.ascontiguousarray(w)

    def prep_b(b, ms, scale=1.0):
        return np.ascontiguousarray(
            (np.asarray(b)[ms] * scale).astype(np.float32).reshape(2, P).T)

    ident = np.eye(P, dtype=ml_dtypes.bfloat16)
    in_maps = []
    for c in range(N_CORES):
        b_idx, g = divmod(c, N_CORES // 2)
        ms = slice(g * M, (g + 1) * M)
        woT = np.ascontiguousarray(Wo[:, ms].T.astype(np.float32))
        in_maps.append({
            "xqT": np.ascontiguousarray(query[b_idx].T.astype(ml_dtypes.bfloat16)),
            "xkT": np.ascontiguousarray(key[b_idx].T.astype(ml_dtypes.bfloat16)),
            "xvT": np.ascontiguousarray(value[b_idx].T.astype(ml_dtypes.bfloat16)),
            "wq": prep_w(Wq, ms, 1.0 / np.sqrt(DK), mt_major=True),
            "wk": prep_w(Wk, ms, mt_major=True),
            "wv": prep_w(Wv, ms),
            "woT": np.ascontiguousarray(
                woT.astype(ml_dtypes.bfloat16).reshape(2, P, D).transpose(1, 0, 2)),
            "bq": prep_b(bq, ms, 1.0 / np.sqrt(DK)),
            "bk": prep_b(bk, ms),
            "bvb": np.ascontiguousarray(
                np.tile(np.asarray(bv)[ms].astype(np.float32), (P, 1))),
            "bo4": np.ascontiguousarray(
                (np.asarray(bo) / (N_CORES // 2)).astype(np.float32)
                .reshape(N_JT, P).T),
            "ident": ident,
        })
    return in_maps


_NC_CACHE = None
_RUNNER_CACHE = None


def _get_nc():
    global _NC_CACHE
    if _NC_CACHE is None:
        _NC_CACHE = build_bass()
    return _NC_CACHE


def _axon_runner():
    """Jit the SPMD NEFF exec once (no donation; kernel writes every output
    element, so reusing non-donated zero buffers across calls is safe)."""
    global _RUNNER_CACHE
    if _RUNNER_CACHE is not None:
        return _RUNNER_CACHE
    import jax
    from jax.experimental.shard_map import shard_map
    from jax.sharding import Mesh, PartitionSpec
    from concourse.bass2jax import (_bass_exec_p, install_neuronx_cc_hook,
                                    partition_id_tensor)

    nc = _get_nc()
    install_neuronx_cc_hook()
    pname = nc.partition_id_tensor.name if nc.partition_id_tensor else None
    in_names, out_names, out_avals = [], [], []
    for alloc in nc.m.functions[0].allocations:
        if not isinstance(alloc, mybir.MemoryLocationSet):
            continue
        name = alloc.memorylocations[0].name
        if alloc.kind == "ExternalInput":
            if name != pname:
                in_names.append(name)
        elif alloc.kind == "ExternalOutput":
            out_names.append(name)
            out_avals.append(jax.core.ShapedArray(
                tuple(alloc.tensor_shape), mybir.dt.np(alloc.dtype)))
    n_params = len(in_names)
    all_names = in_names + out_names
    if pname is not None:
        all_names = all_names + [pname]

    def _body(*args):
        operands = list(args)
        if pname is not None:
            operands.append(partition_id_tensor())
        outs = _bass_exec_p.bind(
            *operands, out_avals=tuple(out_avals), in_names=tuple(all_names),
            out_names=tuple(out_names), lowering_input_output_aliases=(),
            sim_require_finite=True, sim_require_nnan=True, nc=nc)
        return tuple(outs)

    mesh = Mesh(np.asarray(jax.devices()[:N_CORES]), ("core",))
    nin = n_params + len(out_names)
    sharded = jax.jit(
        shard_map(_body, mesh=mesh,
                  in_specs=(PartitionSpec("core"),) * nin,
                  out_specs=(PartitionSpec("core"),) * len(out_names),
                  check_rep=False),
        keep_unused=True)
    zeros = [np.zeros((N_CORES * a.shape[0], *a.shape[1:]), a.dtype)
             for a in out_avals]
    _RUNNER_CACHE = (sharded, in_names, out_names, out_avals, zeros)
    return _RUNNER_CACHE


def _run_axon(in_maps):
    import jax
    sharded, in_names, out_names, out_avals, zeros = _axon_runner()
    concat_in = [
        np.concatenate([np.asarray(in_maps[c][n]) for c in range(N_CORES)],
                       axis=0)
        for n in in_names
    ]
    outs = sharded(*concat_in, *zeros)
    return [
        {n: np.asarray(outs[i]).reshape(N_CORES, *out_avals[i].shape)[c]
         for i, n in enumerate(out_names)}
        for c in range(N_CORES)
    ]


def run(inputs, **kw):
    """Returns (full_output, per-core results list)."""
    from concourse._compat import axon_active

    inputs = {k: np.asarray(v) for k, v in inputs.items()}
    in_maps = shard_inputs(**inputs)
    if axon_active():
        results = _run_axon(in_maps)
    else:
        results = run_bass_kernel_spmd(
            _get_nc(), in_maps, core_ids=list(range(N_CORES)), **kw).results
    B = 2
    out = np.zeros((B, S, D), np.float32)
    for c in range(N_CORES):
        b_idx = c // (N_CORES // 2)
        out[b_idx] += np.asarray(results[c]["outT"]).astype(np.float32).T
    return out, results


def kernel(**inputs):
    out, _ = run(inputs)
    return out
